# revision 22
# baseline (speedup 1.0000x reference)
"""DenseKAN forward as a single fused matmul on TRN2.

Math: the reference uses a uniform knot grid (spacing 0.4 on [-2.2, 2.2]),
so the Cox-de Boor bases are shifted copies of the cardinal cubic B-spline.
Each basis is expanded from the side that keeps the truncated-power
features small:

    right-side blocks n=0..3:  f_n = max(c_n - x, 0)^3,  c_n = (n-1.5)/2.5
    left-side  blocks n=4..7:  f_n = max(x + c_n, 0)^3,  c_n = (5.5-n)/2.5

The silu term is itself approximated by a cubic spline in the same basis
(silu ~= sum_j gamma_j B_j on [-1,1), max abs err 1.9e-5) and folded into
the weights, so no silu is computed on-chip and K drops to 8*256 = 2048.

On-chip per block the cube is built as f = sq * m with
    m  = min(x - c, 0)   (right side; equals -max(c-x,0), sign folded
                          into the weights)  or  max(x + c, 0) (left side)
    sq = m*m (wide DVE op, small blocks) or (x -+ c)^2 via ACT Square
         (large blocks; no clamp needed -- multiplying by m zeroes the
         dead side, and squaring unrounded fp32 x keeps the bf16
         rounding error linear instead of cubed).

All basis coefficients, the per-dim scale factor, and the bias (via
partition of unity) are folded into the bf16 weights on the host. The
host pre-transposes x (ships x^T only, fp32). Batch is sharded across
the 8 cores (128 rows each); weights are replicated.

Block order in the feature tile / weight rows:
    [n0 n1 n6 n7 (small, wide-squared) | n2 n3 n4 n5 (large, ACT)]
Weight rows for right-side blocks (n0..n3) are negated (min-trick).
"""

import numpy as np

import concourse.bass as bass
import concourse.mybir as mybir
import concourse.tile as tile
from concourse import bacc
from concourse.bass_utils import run_bass_kernel_spmd

BATCH = 1024
IN = 256
UNITS = 256
GK = 8  # number of spline bases per input dim
K = IN * GK  # 2048 contraction rows
N_CORES = 8
BS = BATCH // N_CORES  # 128 batch rows per core
KT = K // 128  # 16 K-tiles
N_WARM = 8  # PE warm-up matmuls (HAM clock-gate burn-in)

FP32 = mybir.dt.float32
BF16 = mybir.dt.bfloat16

AluOp = mybir.AluOpType
ActFn = mybir.ActivationFunctionType

# block order: small-|f| blocks first (squared in one wide DVE op),
# large-|f| blocks after (squared on ACT from fp32 x for precision)
BLOCK_ORDER = [0, 1, 6, 7, 2, 3, 4, 5]

# silu ~= sum_j GAMMA[j] * B_j(x) on [-1, 1)  (lstsq fit, err 1.9e-5)
GAMMA = np.array([-0.28180733, -0.27700766, -0.22384863, -0.10329261,
                  0.09670742, 0.37615133, 0.72299243, 1.1181921])


def _block_const(n):
    # returns (c, is_right) such that m = min(x-c,0) [right] / max(x+c,0)
    if n < 4:
        return (n - 1.5) / 2.5, True
    return (5.5 - n) / 2.5, False


def _build():
    nc = bacc.Bacc("TRN2", target_bir_lowering=False, debug=False,
                   enable_asserts=False, num_devices=N_CORES)
    xt_d = nc.dram_tensor("xt", [128, 2 * BS], BF16,
                          kind="ExternalInput").ap()
    # host pre-swizzled: w2[p, k, o] = W2_flat[128*k + p, o], bf16
    w_d = nc.dram_tensor("w2", [128, KT, UNITS], BF16,
                         kind="ExternalInput").ap()
    o_d = nc.dram_tensor("out", [BS, UNITS], BF16, kind="ExternalOutput").ap()

    with tile.TileContext(nc) as tc:
        with (
            tc.tile_pool(name="const", bufs=1) as cpool,
            tc.tile_pool(name="psum", bufs=1, space="PSUM") as ppool,
        ):
            # x first: the whole feature pipeline hangs off it
            xt = cpool.tile([128, 2 * BS], BF16)
            nc.sync.dma_start(xt[:], xt_d[:])

            # weights stream behind x on the same queue (concurrent
            # two-queue streaming slows both via packet round-robin)
            w2 = cpool.tile([128, KT, UNITS], BF16)
            lo = 0
            for sz in (4, 4, 4, 3, 1):
                nc.sync.dma_start(w2[:, lo:lo + sz, :], w_d[:, lo:lo + sz, :])
                lo += sz

            # dummy activation on a const input: hoists the ACT table
            # load (1.28us) ahead of the xt-arrival wait in the scalar
            # queue (walrus emits the PSEUDO_LOAD before the first
            # ACTIVATE, which otherwise sits behind the xt sem wait)
            scratch1 = cpool.tile([128, 1], FP32, tag="actwarm")
            nc.scalar.activation(scratch1[:],
                                 nc.const_aps.tensor(0.0, (128, 1), FP32),
                                 ActFn.Square)

            # PE warm-up: HAM keeps the PE at 1.2 GHz until ~3.4us of
            # sustained activity; burn that in while the weights stream
            wtile = cpool.tile([128, 512], BF16)
            nc.gpsimd.memset(wtile[:], 1.0)
            wpsum = ppool.tile([128, 512], FP32)
            for _ in range(N_WARM):
                nc.tensor.matmul(wpsum[:], wtile[:, 0:128], wtile[:],
                                 start=True, stop=True)

            # per-partition bias tiles for the ACT squares (activation
            # bias must be an AP; only 0/1 are pre-registered consts)
            bias_aps = []
            for i, n in enumerate(BLOCK_ORDER[4:], start=4):
                c, right = _block_const(n)
                bval = -c if right else c
                bt = cpool.tile([128, 1], FP32, tag=f"bias{i}")
                nc.gpsimd.memset(bt[:], float(bval))
                bias_aps.append(bt)

            U = cpool.tile([128, GK * 256], BF16)   # clamped (x -+ c)
            SQ = cpool.tile([128, GK * 256], BF16)  # squares
            T = cpool.tile([128, K], BF16)          # feature tile (lhsT)
            opsum = ppool.tile([BS, UNITS], FP32)

            def clamp(i, n):
                c, right = _block_const(n)
                dst = U[:, i * 256:(i + 1) * 256]
                if right:
                    nc.vector.tensor_scalar(dst, xt[:], float(c), 0.0,
                                            AluOp.subtract, AluOp.min)
                else:
                    nc.vector.tensor_scalar(dst, xt[:], float(c), 0.0,
                                            AluOp.add, AluOp.max)

            # ACT squares for the large blocks straight from fp32 x
            # (sq = (x -+ c)^2, unclamped -- the mul by clamped m zeroes
            # the dead side); ACT runs in parallel with the DVE chain
            for i in range(4, 8):
                nc.scalar.activation(SQ[:, i * 256:(i + 1) * 256], xt[:],
                                     ActFn.Square, bias=bias_aps[i - 4][:])

            # DVE chain, half 0: clamps, wide square, wide cube
            for i in range(4):
                clamp(i, BLOCK_ORDER[i])
            nc.vector.tensor_mul(SQ[:, 0:1024], U[:, 0:1024], U[:, 0:1024])
            nc.vector.tensor_mul(T[:, 0:1024], SQ[:, 0:1024], U[:, 0:1024])
            for k in range(0, 8):
                nc.tensor.matmul(opsum[:], T[:, k * 128:(k + 1) * 128],
                                 w2[:, k, :], start=(k == 0), stop=False)

            # half 1 clamps: two on DVE, two on ACT as Relu(x + c) (left
            # blocks; ACT is idle after the squares, shortens DVE chain)
            for i in (4, 5):
                clamp(i, BLOCK_ORDER[i])
            for i in (6, 7):
                nc.scalar.activation(U[:, i * 256:(i + 1) * 256], xt[:],
                                     ActFn.Relu, bias=bias_aps[i - 4][:])
            nc.vector.tensor_mul(T[:, 1024:2048], SQ[:, 1024:2048],
                                 U[:, 1024:2048])
            for k in range(8, KT):
                nc.tensor.matmul(opsum[:], T[:, k * 128:(k + 1) * 128],
                                 w2[:, k, :], start=False, stop=(k == KT - 1))

            # PSUM -> SBUF (DVE is idle by now; a DVE+ACT split copy gets
            # serialized by the tile-level WAR tracking, so keep one op)
            osb = cpool.tile([BS, UNITS], BF16)
            nc.vector.tensor_copy(osb[:], opsum[:])
            nc.sync.dma_start(o_d[:], osb[:])

    nc.compile()
    return nc


def _fold_weights(spline_kernel, scale_factor, bias):
    """-> (128, KT, UNITS) bf16 swizzled folded weights."""
    sk = spline_kernel.astype(np.float64)
    sf = scale_factor.astype(np.float64)
    b = bias.astype(np.float64)
    # W[i,j,o] = (sk + gamma_j) * sf + bias/IN
    # (silu folded via spline fit, bias via sum_j B_j == 1)
    W = (sk + GAMMA[None, :, None]) * sf[:, None, :] + b[None, None, :] / IN
    comb = 2.5 ** 3 * np.array([1.0, -4.0, 6.0, -4.0, 1.0]) / 6.0
    # A[j, n] = coefficient of feature-block n in basis j
    A = np.zeros((GK, GK))
    for j in range(4):  # right-side: B_j = sum_m comb[m] * f_{j-m}
        for m in range(j + 1):
            A[j, j - m] = comb[m]
    for j in range(4, GK):  # left-side: B_j = sum_m comb[m] * f_{j+m}
        for m in range(GK - j):
            A[j, j + m] = comb[m]
    W2 = np.einsum("jn,ijo->nio", A, W)  # (GK, IN, UNITS)
    # on-chip block order + min-trick sign flip for right-side blocks
    blocks = []
    for n in BLOCK_ORDER:
        wn = W2[n]
        if n < 4:  # right side computed as -t^3 on chip
            wn = -wn
        blocks.append(wn)
    flat = np.stack(blocks, axis=0).reshape(K, UNITS)
    sw = flat.reshape(KT, 128, UNITS).transpose(1, 0, 2)  # -> [p, k, o]
    return np.ascontiguousarray(sw.astype(np.float32)).astype(
        mybir.dt.np(BF16))


def _prep_x(x):
    """(BATCH, IN) -> per-core (128, 2*BS) bf16 SBUF images x^T."""
    x = np.asarray(x, dtype=np.float32)
    outs = []
    for c in range(N_CORES):
        xs = x[c * BS:(c + 1) * BS]  # (BS, IN)
        xtc = np.ascontiguousarray(xs.T)  # (IN, BS)
        b0, b1 = xtc[:128], xtc[128:]
        outs.append(np.ascontiguousarray(
            np.concatenate([b0, b1], axis=1)).astype(
                mybir.dt.np(mybir.dt.bfloat16)))  # (128, 2*BS)
    return outs


def _make_in_maps(inputs):
    w2 = _fold_weights(inputs["spline_kernel"], inputs["scale_factor"],
                       inputs["bias"])
    xts = _prep_x(inputs["x"])
    return [{"xt": xts[c], "w2": w2} for c in range(N_CORES)]


_cache = {}


def kernel(x, spline_kernel, scale_factor, bias):
    if "nc" not in _cache:
        _cache["nc"] = _build()
    nc = _cache["nc"]

    in_maps = _make_in_maps({"x": x, "spline_kernel": spline_kernel,
                             "scale_factor": scale_factor, "bias": bias})
    res = run_bass_kernel_spmd(nc, in_maps, list(range(N_CORES)))
    out = np.concatenate([res.results[c]["out"] for c in range(N_CORES)],
                         axis=0)
    return out.astype(np.float32)
